# revision 1
# baseline (speedup 1.0000x reference)
"""GCN layer (gather -> x@W -> normalized scatter-add -> bias -> PReLU) on 8 trn2 cores.

Strategy (node sharding):
  - 100000 nodes padded to 102400 = 8 * 12800; core c owns nodes [c*12800, (c+1)*12800).
  - Phase 1: each core computes hs = dinv * (x_own @ W) for its nodes (fp32 on PE, x tiles
    transposed via PE), written as bf16 in 4 quarter tensors (3200 rows each).
  - Phase 2: 4 AllGathers (one per quarter) -> 4 shared tables [25600, 128] bf16; each
    pipelines behind the quarter's phase-1 writes and ahead of phase-3 consumers.
  - Phase 3: edges sorted by destination; destinations processed in 128-node windows
    (PSUM [128 dst, 128 feat], 8 windows in flight); per 128-edge block a one-hot
    S [edge, dst] is built on DVE (iota == reldst) and PE accumulates psum += S^T @ G,
    where G = dma_gather'ed hs rows (int16 idx into the 25600-row quarter table).
    Self-loops are one identity matmul per window on contiguous hs rows.
    Epilogue: out = prelu(dinv_dst * psum + b) with per-feature alpha.
"""
import sys
sys.path.insert(0, '/opt/trn_rl_repo')

import numpy as np
import ml_dtypes

N = 100000
NCORES = 8
SH = 12800                 # nodes per core
NP = NCORES * SH           # 102400 padded nodes
H = 128                    # output features
KIN = 256                  # input features
WIN = 128                  # dst window size
NW = SH // WIN             # 100 windows per core
WG = 8                     # windows per PSUM group
NG = (NW + WG - 1) // WG   # 13 groups (last has 4)
NQ = 4                     # source quarters
QSH = SH // NQ             # 3200 rows of own shard per quarter
QT = QSH // WIN            # 25 tiles per quarter
TAB = NCORES * QSH         # 25600 rows per gather table (int16-safe)
XB = 5                     # phase-1 tiles per DMA batch (25 tiles/quarter = 5 batches)

bf16 = ml_dtypes.bfloat16


def _preprocess(edge_index):
    src = np.asarray(edge_index[0]).astype(np.int64)
    dst = np.asarray(edge_index[1]).astype(np.int64)
    E = src.shape[0]

    deg = (np.bincount(dst, minlength=N) + 1).astype(np.float32)
    dinv = (1.0 / np.sqrt(deg)).astype(np.float32)
    dinv_np = np.ones(NP, np.float32)
    dinv_np[:N] = dinv

    core = dst // SH
    w_in_core = (dst % SH) // WIN            # 0..NW-1
    g = w_in_core // WG
    wi = w_in_core % WG
    # source quarter + row in its gather table
    s_core = src // SH
    s_li = src % SH
    q = s_li // QSH
    tab_row = s_core * QSH + (s_li % QSH)    # < TAB

    key = ((core * NG + g) * NQ + q) * WG + wi
    nbins_pc = NG * NQ * WG
    order = np.argsort(key, kind='stable')
    o_tab = tab_row[order]
    o_dst = dst[order]
    o_key = key[order]

    cnt_all = np.bincount(key, minlength=NCORES * nbins_pc)
    bin_start = np.concatenate([[0], np.cumsum(cnt_all)])[:-1]
    rank = np.arange(E, dtype=np.int64) - bin_start[o_key]

    cnt = cnt_all.reshape(NCORES, NG, NQ, WG)
    nblk = np.ceil(cnt.max(axis=0) / WIN).astype(np.int64)   # [NG, NQ, WG] common
    pad_sizes = (nblk * WIN).reshape(-1)
    offs = np.concatenate([[0], np.cumsum(pad_sizes)])
    TOT = int(offs[-1])
    NBLK = TOT // WIN

    bin_in_core = o_key % nbins_pc
    pos = offs[bin_in_core] + rank
    win_base = core * SH + w_in_core * WIN
    o_win_base = win_base[order]
    o_core = core[order]

    per_core = []
    for c in range(NCORES):
        m = o_core == c
        idxq = np.zeros(TOT, np.int16)
        rels = np.full(TOT, -1.0, np.float32)
        p_c = pos[m]
        idxq[p_c] = o_tab[m].astype(np.int16)
        rels[p_c] = (o_dst[m] - o_win_base[m]).astype(np.float32)
        idx16 = np.tile(np.ascontiguousarray(idxq.reshape(TOT // 16, 16).T), (8, 1))
        relm = np.ascontiguousarray(rels.reshape(NBLK, WIN).T)   # [128, NBLK]
        dinv_own = np.ascontiguousarray(
            dinv_np[c * SH:(c + 1) * SH].reshape(NW, WIN).T)     # [128, NW]
        per_core.append(dict(idx16=idx16, reldst=relm, dinv=dinv_own))

    calls = []          # (g, q, off_idx, nidx, [(Bcol, w), ...])
    Bcol = 0
    last_block_of_win = {}
    for gg in range(NG):
        for qq in range(NQ):
            blocks = []
            off_idx = None
            for wii in range(WG):
                nb = int(nblk[gg, qq, wii])
                if nb == 0:
                    continue
                w = gg * WG + wii
                if w >= NW:
                    continue
                bin_i = (gg * NQ + qq) * WG + wii
                if off_idx is None:
                    off_idx = int(offs[bin_i])
                for k in range(nb):
                    blocks.append((Bcol, w))
                    last_block_of_win[w] = Bcol
                    Bcol += 1
            if blocks:
                calls.append((gg, qq, off_idx, len(blocks) * WIN, blocks))
    sched = dict(calls=calls, last_block=last_block_of_win, NBLK=NBLK, TOT=TOT)
    return sched, per_core, dinv_np


def _build(sched):
    import os
    from concourse import bass, bacc, tile, mybir
    from concourse.masks import make_identity

    nc = bacc.Bacc("TRN2", target_bir_lowering=False, debug=False,
                   enable_asserts=True, num_devices=NCORES)

    x_d = nc.dram_tensor("x_own", [SH, KIN], mybir.dt.float32, kind="ExternalInput")
    w_d = nc.dram_tensor("w_mat", [KIN, H], mybir.dt.float32, kind="ExternalInput")
    b_d = nc.dram_tensor("b_vec", [H], mybir.dt.float32, kind="ExternalInput")
    a_d = nc.dram_tensor("a_vec", [H], mybir.dt.float32, kind="ExternalInput")
    dinv_d = nc.dram_tensor("dinv_own", [128, NW], mybir.dt.float32, kind="ExternalInput")
    idx_d = nc.dram_tensor("idx16", [128, sched["TOT"] // 16], mybir.dt.int16, kind="ExternalInput")
    rel_d = nc.dram_tensor("reldst", [128, sched["NBLK"]], mybir.dt.float32, kind="ExternalInput")

    out_d = nc.dram_tensor("out_own", [SH, H], mybir.dt.float32, kind="ExternalOutput")

    hs_q = [nc.dram_tensor(f"hs_q{k}", [QSH, H], mybir.dt.bfloat16) for k in range(NQ)]
    hs_tab = [nc.dram_tensor(f"hs_tab{k}", [TAB, H], mybir.dt.bfloat16, addr_space="Shared")
              for k in range(NQ)]

    calls = sched["calls"]
    last_block = sched["last_block"]
    max_call_blk = max(len(cb[4]) for cb in calls)
    nblk_of_group = [sum(len(cb[4]) for cb in calls if cb[0] == gg) for gg in range(NG)]
    first_col_of_group = [min([cb[4][0][0] for cb in calls if cb[0] == gg] or [0])
                          for gg in range(NG)]

    with tile.TileContext(nc) as tc:
        with tc.tile_pool(name="consts", bufs=1) as cp, tc.tile_pool(name="sb", bufs=3) as sb:
            # ---------------- constants ----------------
            iota_i = cp.tile([128, 128], mybir.dt.int32)
            nc.gpsimd.iota(iota_i[:], pattern=[[1, 128]], base=0, channel_multiplier=0)
            iota_f = cp.tile([128, 128], mybir.dt.float32)
            nc.vector.tensor_copy(iota_f[:], iota_i[:])
            iota_b = cp.tile([128, 128], mybir.dt.bfloat16)
            nc.vector.tensor_copy(iota_b[:], iota_i[:])

            ident_f = cp.tile([128, 128], mybir.dt.float32)
            make_identity(nc, ident_f[:])
            ident_b = cp.tile([128, 128], mybir.dt.bfloat16)
            nc.vector.tensor_copy(ident_b[:], ident_f[:])

            w0 = cp.tile([128, H], mybir.dt.float32)
            w1 = cp.tile([128, H], mybir.dt.float32)
            nc.sync.dma_start(w0[:], w_d[0:128, :])
            nc.sync.dma_start(w1[:], w_d[128:256, :])

            dinv_sb = cp.tile([128, NW], mybir.dt.float32)
            nc.sync.dma_start(dinv_sb[:], dinv_d[:, :])

            ones1 = cp.tile([1, H], mybir.dt.float32)
            nc.vector.memset(ones1[:], 1.0)
            bvec = cp.tile([1, H], mybir.dt.float32)
            nc.sync.dma_start(bvec[:], b_d[None, :])
            avec = cp.tile([1, H], mybir.dt.float32)
            nc.sync.dma_start(avec[:], a_d[None, :])

            b128 = cp.tile([128, H], mybir.dt.float32)
            a128 = cp.tile([128, H], mybir.dt.float32)
            hs_sb = cp.tile([128, NW * H], mybir.dt.bfloat16)   # own hs, SBUF-resident

            with tc.tile_pool(name="psum1", bufs=1, space="PSUM") as pp1:
                bc_ps = pp1.tile([128, H], mybir.dt.float32, space="PSUM", tag="bc", bufs=1)
                nc.tensor.matmul(out=bc_ps[:], lhsT=ones1[:], rhs=bvec[:], start=True, stop=True)
                nc.vector.tensor_copy(b128[:], bc_ps[:])
                ac_ps = pp1.tile([128, H], mybir.dt.float32, space="PSUM", tag="bc", bufs=1)
                nc.tensor.matmul(out=ac_ps[:], lhsT=ones1[:], rhs=avec[:], start=True, stop=True)
                nc.vector.tensor_copy(a128[:], ac_ps[:])

                # ---------------- phase 1 (+ per-quarter AllGather) ----------------
                for bb in range(NW // XB):          # batches of XB tiles
                    qk = bb // (QT // XB)           # quarter of this batch
                    t0 = bb * XB
                    x_t = sb.tile([128, XB * KIN], mybir.dt.float32, tag="x_t", bufs=3)
                    nc.sync.dma_start(
                        x_t[:],
                        x_d[t0 * 128:(t0 + XB) * 128, :].rearrange(
                            "(t p) k -> p t k", p=128))
                    hs_b = hs_sb[:, t0 * H:(t0 + XB) * H]
                    for tt in range(XB):
                        i = t0 + tt
                        h_ps = pp1.tile([128, H], mybir.dt.float32, space="PSUM",
                                        tag="h_ps", bufs=3)
                        for kk in range(2):
                            xt_ps = pp1.tile([128, 128], mybir.dt.float32, space="PSUM",
                                             tag="xt_ps", bufs=4)
                            nc.tensor.transpose(
                                xt_ps[:], x_t[:, (tt * 2 + kk) * 128:(tt * 2 + kk + 1) * 128],
                                ident_f[:])
                            xt_sb = sb.tile([128, 128], mybir.dt.float32, tag="xt_sb", bufs=4)
                            nc.vector.tensor_copy(xt_sb[:], xt_ps[:])
                            nc.tensor.matmul(out=h_ps[:], lhsT=xt_sb[:],
                                             rhs=(w0 if kk == 0 else w1)[:],
                                             start=(kk == 0), stop=(kk == 1))
                        nc.scalar.activation(hs_b[:, tt * H:(tt + 1) * H], h_ps[:],
                                             mybir.ActivationFunctionType.Copy,
                                             scale=dinv_sb[:, i:i + 1])
                    r0 = t0 * 128 - qk * QSH
                    nc.sync.dma_start(
                        hs_q[qk][r0:r0 + XB * 128, :].rearrange("(t p) k -> p t k", p=128),
                        hs_b)
                    if (bb + 1) % (QT // XB) == 0:
                        if os.environ.get("K_FAKE_COLL"):
                            # timing probe only: dependency-equivalent local copy
                            nc.sync.dma_start(hs_tab[qk][0:QSH, :], hs_q[qk][:, :])
                        else:
                            nc.gpsimd.collective_compute(
                                "AllGather", mybir.AluOpType.bypass,
                                replica_groups=[list(range(NCORES))],
                                ins=[hs_q[qk].ap().opt()],
                                outs=[hs_tab[qk].ap().opt()],
                            )

            # ---------------- phase 3 ----------------
            with tc.tile_pool(name="psum3", bufs=WG, space="PSUM") as pp3:
                for gg in range(NG):
                    wlo = gg * WG
                    whi = min(wlo + WG, NW)
                    nwin = whi - wlo
                    pw = {}
                    for w in range(wlo, whi):
                        pwt = pp3.tile([128, H], mybir.dt.float32, space="PSUM",
                                       tag="pw", name=f"pw{w}", bufs=8)
                        pw[w] = pwt[:]
                        nc.tensor.matmul(out=pw[w], lhsT=ident_b[:],
                                         rhs=hs_sb[:, w * H:(w + 1) * H],
                                         start=True, stop=(w not in last_block))

                    if nblk_of_group[gg]:
                        rd_sb = sb.tile([128, max(nblk_of_group)], mybir.dt.float32,
                                        tag="rd", bufs=3)
                        c0 = first_col_of_group[gg]
                        nc.sync.dma_start(rd_sb[:, 0:nblk_of_group[gg]],
                                          rel_d[:, c0:c0 + nblk_of_group[gg]])

                    for (g_c, qq, off_idx, nidx, blocks) in calls:
                        if g_c != gg:
                            continue
                        idx_sb = sb.tile([128, max_call_blk * 8], mybir.dt.int16,
                                         tag="idx", bufs=4)
                        nc.sync.dma_start(idx_sb[:, 0:nidx // 16],
                                          idx_d[:, off_idx // 16: (off_idx + nidx) // 16])
                        g_t = sb.tile([128, max_call_blk, H], mybir.dt.bfloat16,
                                      tag="g_t", bufs=5)
                        nc.gpsimd.dma_gather(
                            g_t[:, 0:nidx // 128, :], hs_tab[qq][:, :],
                            idx_sb[:, 0:nidx // 16], nidx, nidx, H,
                            single_packet=False)
                        for (bcol, w) in blocks:
                            s_t = sb.tile([128, 128], mybir.dt.bfloat16, tag="s_t", bufs=8)
                            lc = bcol - first_col_of_group[gg]
                            nc.vector.tensor_scalar(
                                out=s_t[:], in0=iota_b[:],
                                scalar1=rd_sb[:, lc:lc + 1], scalar2=None,
                                op0=mybir.AluOpType.is_equal)
                            slot = (bcol - blocks[0][0])
                            nc.tensor.matmul(out=pw[w], lhsT=s_t[:], rhs=g_t[:, slot, :],
                                             start=False, stop=(last_block.get(w) == bcol))

                    # epilogue, batched output DMA per group
                    o_g = sb.tile([128, WG * H], mybir.dt.float32, tag="o_g", bufs=2)
                    for w in range(wlo, whi):
                        u = sb.tile([128, H], mybir.dt.float32, tag="u", bufs=4)
                        nc.scalar.activation(u[:], pw[w], mybir.ActivationFunctionType.Copy,
                                             scale=dinv_sb[:, w:w + 1])
                        u2 = sb.tile([128, H], mybir.dt.float32, tag="u2", bufs=4)
                        nc.vector.tensor_tensor(out=u2[:], in0=u[:], in1=b128[:],
                                                op=mybir.AluOpType.add)
                        r2 = sb.tile([128, H], mybir.dt.float32, tag="r2", bufs=3)
                        nc.scalar.activation(r2[:], u2[:], mybir.ActivationFunctionType.Relu,
                                             scale=-1.0)
                        m = sb.tile([128, H], mybir.dt.float32, tag="m", bufs=3)
                        nc.gpsimd.tensor_tensor(out=m[:], in0=r2[:], in1=a128[:],
                                                op=mybir.AluOpType.mult)
                        r1 = sb.tile([128, H], mybir.dt.float32, tag="r1", bufs=3)
                        nc.scalar.activation(r1[:], u2[:], mybir.ActivationFunctionType.Relu)
                        nc.vector.tensor_tensor(out=o_g[:, (w - wlo) * H:(w - wlo + 1) * H],
                                                in0=r1[:], in1=m[:],
                                                op=mybir.AluOpType.subtract)
                    nc.sync.dma_start(
                        out_d[wlo * 128:whi * 128, :].rearrange("(t p) k -> p t k", p=128),
                        o_g[:, 0:nwin * H])

    nc.compile()
    return nc


_LAST = {}


def kernel(x, edge_index, W, b, alpha):
    from concourse.bass_utils import run_bass_kernel_spmd

    x = np.asarray(x, dtype=np.float32)
    W = np.asarray(W, dtype=np.float32)
    b = np.asarray(b, dtype=np.float32)
    alpha = np.asarray(alpha, dtype=np.float32)

    sched, per_core, dinv_np = _preprocess(edge_index)
    nc = _build(sched)
    _LAST["nc"] = nc
    _LAST["sched"] = sched

    x_pad = np.zeros((NP, KIN), np.float32)
    x_pad[:N] = x

    in_maps = []
    for c in range(NCORES):
        in_maps.append({
            "x_own": np.ascontiguousarray(x_pad[c * SH:(c + 1) * SH]),
            "w_mat": W, "b_vec": b, "a_vec": alpha,
            "dinv_own": per_core[c]["dinv"],
            "idx16": per_core[c]["idx16"],
            "reldst": per_core[c]["reldst"],
        })

    res = run_bass_kernel_spmd(nc, in_maps, core_ids=list(range(NCORES)))
    out = np.concatenate([res.results[c]["out_own"] for c in range(NCORES)], axis=0)
    return np.ascontiguousarray(out[:N])



# revision 7
# speedup vs baseline: 2.1531x; 2.1531x over previous
"""GCN layer (gather -> aggregate -> @W -> bias -> PReLU) on 8 trn2 cores.

Strategy (aggregate-then-transform, collective-free):
  - out = PReLU(dinv_dst * (A @ xs) @ W + b), xs = x * dinv_src (bf16, host-precomputed).
  - The full xs table [102400, 256] bf16 is replicated to every core as 4 quarter
    tables of 25600 rows (dma_gather indices are int16). No collectives at all.
  - Core c owns dst nodes [c*12800, (c+1)*12800): 100 windows of 128 dst, 13 groups
    of 8 windows. Edges are sorted by (dst_core, group, src_quarter, window) and
    padded to 128-edge blocks only at (group, quarter) boundaries (~6% pad).
  - Per block: dma_gather 128 xs rows (512B descriptors: full DMA rate), build
    one-hot S[e, d] = (iota == rel) on DVE per overlapped window, and accumulate
    aggT[k, d] += G^T @ S on PE into per-window PSUM ([128,128] x 2 feature halves).
    Self-loops are one identity matmul per window half on contiguous xs_own rows.
  - Group tail: copy aggT to SBUF bf16, pw = aggT^T @ W + sqrt(deg) x b (outer
    product), epilogue out = relu(dinv*pw) - alpha*relu(-dinv*pw), batched out DMA.
"""
import sys
sys.path.insert(0, '/opt/trn_rl_repo')

import numpy as np
import ml_dtypes

N = 100000
NCORES = 8
SH = 12800                 # dst nodes per core
NP = NCORES * SH           # 102400 padded nodes
H = 128                    # output features
KIN = 256                  # input features
WIN = 128                  # dst window size
NW = SH // WIN             # 100 windows per core
WG = 8                     # windows per PSUM group
NG = (NW + WG - 1) // WG   # 13 groups (last has 4)
NQ = 4                     # source quarter tables
QROWS = NP // NQ           # 25600 rows per table (int16-safe)
CB = 32                    # max blocks per dma_gather call

bf16 = ml_dtypes.bfloat16


def _preprocess(edge_index):
    src = np.asarray(edge_index[0]).astype(np.int64)
    dst = np.asarray(edge_index[1]).astype(np.int64)
    E = src.shape[0]

    deg = (np.bincount(dst, minlength=N) + 1).astype(np.float32)
    dinv = (1.0 / np.sqrt(deg)).astype(np.float32)
    dinv_np = np.ones(NP, np.float32)
    dinv_np[:N] = dinv
    sqdeg_np = (1.0 / dinv_np).astype(np.float32)

    core = dst // SH
    w_in_core = (dst % SH) // WIN            # 0..NW-1
    g = w_in_core // WG
    q = src // QROWS
    tabrow = (src % QROWS).astype(np.int64)
    rel_in_grp = (dst % SH - g * (WG * WIN)).astype(np.float32)   # 0..1023

    binid = (core * NG + g) * NQ + q         # per-core bin = (g, q)
    nbins_pc = NG * NQ
    order = np.lexsort((w_in_core, binid))
    o_bin = binid[order]
    o_tab = tabrow[order]
    o_rel = rel_in_grp[order]
    o_core = core[order]

    cnt_all = np.bincount(binid, minlength=NCORES * nbins_pc)
    bin_start = np.concatenate([[0], np.cumsum(cnt_all)])[:-1]
    rank = np.arange(E, dtype=np.int64) - bin_start[o_bin]

    cnt = cnt_all.reshape(NCORES, NG, NQ)
    nblk = np.ceil(cnt.max(axis=0) / WIN).astype(np.int64)        # [NG, NQ]
    pad_sizes = (nblk * WIN).reshape(-1)
    offs = np.concatenate([[0], np.cumsum(pad_sizes)])
    TOT = int(offs[-1])
    NBLK = TOT // WIN

    bin_in_core = o_bin % nbins_pc
    pos = offs[bin_in_core] + rank

    # per-core slot tables
    idx_pc = np.zeros((NCORES, TOT), np.int16)
    rel_pc = np.full((NCORES, TOT), -1.0, np.float32)
    for c in range(NCORES):
        m = o_core == c
        p_c = pos[m]
        idx_pc[c, p_c] = o_tab[m].astype(np.int16)
        rel_pc[c, p_c] = o_rel[m]

    # union window range per block (over cores)
    win_pc = np.where(rel_pc >= 0, rel_pc // WIN, np.nan).reshape(NCORES, NBLK, WIN)
    with np.errstate(all="ignore"):
        wlo = np.nanmin(win_pc, axis=(0, 2)).astype(np.int64)     # [NBLK]
        whi = np.nanmax(win_pc, axis=(0, 2)).astype(np.int64)

    # schedule: groups -> calls (per quarter, <=CB blocks) -> blocks -> pairs
    # pair columns of relp are allocated in issue order.
    groups = []       # per g: dict(calls=[...], last=..)
    pair_cols = []    # per pair: (block_global, wl) to build relp
    last_pair_of_win = {}
    npair = 0
    blk_base = 0
    for gg in range(NG):
        calls = []
        for qq in range(NQ):
            bi = gg * NQ + qq
            nb = int(nblk[gg, qq])
            b0 = int(offs[bi]) // WIN
            done = 0
            while done < nb:
                nbc = min(CB, nb - done)
                blocks = []
                for k in range(nbc):
                    bg = b0 + done + k
                    pairs = []
                    for wl in range(int(wlo[bg]), int(whi[bg]) + 1):
                        w = gg * WG + wl
                        pairs.append((wl, npair))
                        last_pair_of_win[w] = npair
                        pair_cols.append((bg, wl))
                        npair += 1
                    blocks.append((k, pairs))
                calls.append(dict(q=qq, off=int(offs[bi]) + done * WIN,
                                  nidx=nbc * WIN, blocks=blocks))
                done += nbc
        groups.append(calls)
        blk_base += 0
    NPAIR = npair

    # per-core device tensors
    per_core = []
    for c in range(NCORES):
        idx16 = np.tile(np.ascontiguousarray(
            idx_pc[c].reshape(TOT // 16, 16).T), (8, 1))          # [128, TOT//16]
        relp = np.empty((WIN, NPAIR), np.float32)                 # [128, NPAIR]
        relb = rel_pc[c].reshape(NBLK, WIN)
        for p, (bg, wl) in enumerate(pair_cols):
            relp[:, p] = relb[bg] - wl * WIN
        dinv_own = np.ascontiguousarray(
            dinv_np[c * SH:(c + 1) * SH].reshape(NW, WIN).T)      # [128, NW]
        sq_own = np.ascontiguousarray(
            sqdeg_np[c * SH:(c + 1) * SH].astype(bf16)[None, :])  # [1, SH]
        per_core.append(dict(idx16=idx16, relp=np.ascontiguousarray(relp),
                             dinv=dinv_own, dinvn=np.ascontiguousarray(-dinv_own),
                             sq=sq_own))

    sched = dict(groups=groups, last=last_pair_of_win, TOT=TOT, NPAIR=NPAIR)
    return sched, per_core, dinv_np


def _build(sched):
    from concourse import bass, bacc, tile, mybir
    from concourse.masks import make_identity

    nc = bacc.Bacc("TRN2", target_bir_lowering=False, debug=False,
                   enable_asserts=True, num_devices=NCORES)

    TOT = sched["TOT"]
    NPAIR = sched["NPAIR"]
    groups = sched["groups"]
    last = sched["last"]

    xs_q = [nc.dram_tensor(f"xs_q{k}", [QROWS, KIN], mybir.dt.bfloat16,
                           kind="ExternalInput") for k in range(NQ)]
    xso_d = nc.dram_tensor("xs_own", [SH, KIN], mybir.dt.bfloat16, kind="ExternalInput")
    w_d = nc.dram_tensor("w_mat", [KIN, H], mybir.dt.bfloat16, kind="ExternalInput")
    b_d = nc.dram_tensor("b_row", [1, H], mybir.dt.bfloat16, kind="ExternalInput")
    a_d = nc.dram_tensor("a128", [128, H], mybir.dt.float32, kind="ExternalInput")
    dinv_d = nc.dram_tensor("dinv_own", [128, NW], mybir.dt.float32, kind="ExternalInput")
    dinvn_d = nc.dram_tensor("dinvn_own", [128, NW], mybir.dt.float32, kind="ExternalInput")
    sq_d = nc.dram_tensor("sq_own", [1, SH], mybir.dt.bfloat16, kind="ExternalInput")
    idx_d = nc.dram_tensor("idx16", [128, TOT // 16], mybir.dt.int16, kind="ExternalInput")
    relp_d = nc.dram_tensor("relp", [128, NPAIR], mybir.dt.float32, kind="ExternalInput")

    out_d = nc.dram_tensor("out_own", [SH, H], mybir.dt.float32, kind="ExternalOutput")

    with tile.TileContext(nc) as tc:
        with tc.tile_pool(name="consts", bufs=1) as cp, tc.tile_pool(name="sb", bufs=3) as sb:
            iota_i = cp.tile([128, 128], mybir.dt.int32)
            nc.gpsimd.iota(iota_i[:], pattern=[[1, 128]], base=0, channel_multiplier=0)
            iota_b = cp.tile([128, 128], mybir.dt.bfloat16)
            nc.vector.tensor_copy(iota_b[:], iota_i[:])

            ident_f = cp.tile([128, 128], mybir.dt.float32)
            make_identity(nc, ident_f[:])
            ident_b = cp.tile([128, 128], mybir.dt.bfloat16)
            nc.vector.tensor_copy(ident_b[:], ident_f[:])

            w0 = cp.tile([128, H], mybir.dt.bfloat16)
            w1 = cp.tile([128, H], mybir.dt.bfloat16)
            nc.sync.dma_start(w0[:], w_d[0:128, :])
            nc.sync.dma_start(w1[:], w_d[128:256, :])

            dinv_sb = cp.tile([128, NW], mybir.dt.float32)
            nc.sync.dma_start(dinv_sb[:], dinv_d[:, :])
            dinvn_sb = cp.tile([128, NW], mybir.dt.float32)
            nc.sync.dma_start(dinvn_sb[:], dinvn_d[:, :])

            a128 = cp.tile([128, H], mybir.dt.float32)
            nc.sync.dma_start(a128[:], a_d[:, :])
            brow = cp.tile([1, H], mybir.dt.bfloat16)
            nc.sync.dma_start(brow[:], b_d[:, :])
            sq_sb = cp.tile([1, SH], mybir.dt.bfloat16)
            nc.sync.dma_start(sq_sb[:], sq_d[:, :])
            relp_sb = cp.tile([128, NPAIR], mybir.dt.float32)
            nc.sync.dma_start(relp_sb[:], relp_d[:, :])

            with tc.tile_pool(name="psum", bufs=1, space="PSUM") as pp:
                for gg in range(NG):
                    wlo_g = gg * WG
                    whi_g = min(wlo_g + WG, NW)
                    nwin = whi_g - wlo_g

                    xso_g = sb.tile([128, WG, KIN], mybir.dt.bfloat16, tag="xso", bufs=2)
                    nc.sync.dma_start(
                        xso_g[:, 0:nwin, :],
                        xso_d[wlo_g * WIN:whi_g * WIN, :].rearrange(
                            "(t p) k -> p t k", p=128))

                    # aggT regions packed 4-per-PSUM-bank: tile [128,512] fp32.
                    # One accumulation group per 2KB zero region (= bank): only the
                    # first matmul issued into a bank starts (zeroing the whole
                    # bank), only the last one stops.
                    nbk = (2 * nwin + 3) // 4
                    agbank = [pp.tile([128, 512], mybir.dt.float32, space="PSUM",
                                      tag="ag", name=f"ag{gg}_{j}", bufs=4)
                              for j in range(nbk)]
                    bank_stop = {}     # bank j -> pcol of its stop pair (or None)
                    for j in range(nbk):
                        ws = [wlo_g + r // 2 for r in range(j * 4, min(j * 4 + 4, 2 * nwin))]
                        pcs = [last[w] for w in set(ws) if w in last]
                        bank_stop[j] = max(pcs) if pcs else None
                    ag = {}
                    bank_of = {}
                    for w in range(wlo_g, whi_g):
                        wl = w - wlo_g
                        for h in range(2):
                            ridx = wl * 2 + h
                            j = ridx // 4
                            ag[(w, h)] = agbank[j][:, (ridx % 4) * 128:
                                                   (ridx % 4) * 128 + 128]
                            bank_of[(w, h)] = j
                            is_last_self = (bank_stop[j] is None
                                            and ridx == min(j * 4 + 3, 2 * nwin - 1))
                            nc.tensor.matmul(
                                out=ag[(w, h)],
                                lhsT=xso_g[:, wl, h * 128:(h + 1) * 128],
                                rhs=ident_b[:],
                                start=(ridx % 4 == 0), stop=is_last_self)

                    for call in groups[gg]:
                        qq, off, nidx = call["q"], call["off"], call["nidx"]
                        nbc = nidx // WIN
                        idx_sb = sb.tile([128, CB * 8], mybir.dt.int16, tag="idx", bufs=4)
                        nc.sync.dma_start(idx_sb[:, 0:nidx // 16],
                                          idx_d[:, off // 16:(off + nidx) // 16])
                        g_t = sb.tile([128, CB, KIN], mybir.dt.bfloat16, tag="g_t", bufs=3)
                        nc.gpsimd.dma_gather(
                            g_t[:, 0:nbc, :], xs_q[qq][:, :],
                            idx_sb[:, 0:nidx // 16], nidx, nidx, KIN,
                            single_packet=False)
                        for (slot, pairs) in call["blocks"]:
                            for (wl, pcol) in pairs:
                                w = wlo_g + wl
                                s_t = sb.tile([128, 128], mybir.dt.bfloat16,
                                              tag="s_t", bufs=8)
                                nc.vector.tensor_scalar(
                                    out=s_t[:], in0=iota_b[:],
                                    scalar1=relp_sb[:, pcol:pcol + 1], scalar2=None,
                                    op0=mybir.AluOpType.is_equal)
                                for h in range(2):
                                    nc.tensor.matmul(
                                        out=ag[(w, h)],
                                        lhsT=g_t[:, slot, h * 128:(h + 1) * 128],
                                        rhs=s_t[:],
                                        start=False,
                                        stop=(bank_stop[bank_of[(w, h)]] == pcol
                                              and h == 1))

                    # group tail: transform + bias + PReLU epilogue
                    o_g = sb.tile([128, WG * H], mybir.dt.float32, tag="o_g", bufs=2)
                    npwb = (nwin + 3) // 4
                    pwbank = [pp.tile([128, 512], mybir.dt.float32, space="PSUM",
                                      tag="pw", name=f"pw{gg}_{j}", bufs=2)
                              for j in range(npwb)]
                    for w in range(wlo_g, whi_g):
                        wl = w - wlo_g
                        u0 = sb.tile([128, 128], mybir.dt.bfloat16, tag="u", bufs=4)
                        nc.scalar.activation(u0[:], ag[(w, 0)],
                                             mybir.ActivationFunctionType.Copy)
                        u1 = sb.tile([128, 128], mybir.dt.bfloat16, tag="u", bufs=4)
                        nc.scalar.activation(u1[:], ag[(w, 1)],
                                             mybir.ActivationFunctionType.Copy)
                        pw = pwbank[wl // 4][:, (wl % 4) * H:(wl % 4) * H + H]
                        pw_first = (wl % 4 == 0)
                        pw_last = (wl == nwin - 1) or (wl % 4 == 3)
                        nc.tensor.matmul(out=pw, lhsT=u0[:], rhs=w0[:],
                                         start=pw_first, stop=False)
                        nc.tensor.matmul(out=pw, lhsT=u1[:], rhs=w1[:],
                                         start=False, stop=False)
                        nc.tensor.matmul(out=pw,
                                         lhsT=sq_sb[:, w * WIN:(w + 1) * WIN],
                                         rhs=brow[:], start=False, stop=pw_last)
                        r1 = sb.tile([128, H], mybir.dt.float32, tag="r1", bufs=3)
                        nc.scalar.activation(r1[:], pw,
                                             mybir.ActivationFunctionType.Relu,
                                             scale=dinv_sb[:, w:w + 1])
                        r2 = sb.tile([128, H], mybir.dt.float32, tag="r2", bufs=3)
                        nc.scalar.activation(r2[:], pw,
                                             mybir.ActivationFunctionType.Relu,
                                             scale=dinvn_sb[:, w:w + 1])
                        m = sb.tile([128, H], mybir.dt.float32, tag="m", bufs=3)
                        nc.gpsimd.tensor_tensor(out=m[:], in0=r2[:], in1=a128[:],
                                                op=mybir.AluOpType.mult)
                        nc.vector.tensor_tensor(out=o_g[:, wl * H:(wl + 1) * H],
                                                in0=r1[:], in1=m[:],
                                                op=mybir.AluOpType.subtract)
                    nc.sync.dma_start(
                        out_d[wlo_g * WIN:whi_g * WIN, :].rearrange(
                            "(t p) k -> p t k", p=128),
                        o_g[:, 0:nwin * H])

    nc.compile()
    return nc


_LAST = {}


def kernel(x, edge_index, W, b, alpha):
    from concourse.bass_utils import run_bass_kernel_spmd

    x = np.asarray(x, dtype=np.float32)
    W = np.asarray(W, dtype=np.float32)
    b = np.asarray(b, dtype=np.float32)
    alpha = np.asarray(alpha, dtype=np.float32)

    sched, per_core, dinv_np = _preprocess(edge_index)
    nc = _build(sched)
    _LAST["nc"] = nc
    _LAST["sched"] = sched

    xs_pad = np.zeros((NP, KIN), np.float32)
    xs_pad[:N] = x * dinv_np[:N, None]
    xs_bf = xs_pad.astype(bf16)
    xsq = [np.ascontiguousarray(xs_bf[k * QROWS:(k + 1) * QROWS]) for k in range(NQ)]

    W_bf = W.astype(bf16)
    b_row = b.astype(bf16)[None, :]
    a128 = np.tile(alpha[None, :], (128, 1)).astype(np.float32)

    in_maps = []
    for c in range(NCORES):
        in_maps.append({
            "xs_q0": xsq[0], "xs_q1": xsq[1], "xs_q2": xsq[2], "xs_q3": xsq[3],
            "xs_own": np.ascontiguousarray(xs_bf[c * SH:(c + 1) * SH]),
            "w_mat": W_bf, "b_row": b_row, "a128": a128,
            "dinv_own": per_core[c]["dinv"], "dinvn_own": per_core[c]["dinvn"],
            "sq_own": per_core[c]["sq"],
            "idx16": per_core[c]["idx16"], "relp": per_core[c]["relp"],
        })

    res = run_bass_kernel_spmd(nc, in_maps, core_ids=list(range(NCORES)))
    out = np.concatenate([res.results[c]["out_own"] for c in range(NCORES)], axis=0)
    return np.ascontiguousarray(out[:N])


# revision 10
# speedup vs baseline: 2.2632x; 1.0512x over previous
"""GCN layer (gather -> aggregate -> @W -> bias -> PReLU) on 8 trn2 cores.

Strategy (aggregate-then-transform, collective-free):
  - out = PReLU(dinv_dst * (A @ xs) @ W + b), xs = x * dinv_src (bf16, host-precomputed).
  - The full xs table [102400, 256] bf16 is replicated to every core as 4 quarter
    tables of 25600 rows (dma_gather indices are int16). No collectives at all.
  - Core c owns dst nodes [c*12800, (c+1)*12800): 100 windows of 128 dst, 13 groups
    of 8 windows. Edges are sorted by (dst_core, group, src_quarter, window) and
    padded to 128-edge blocks only at (group, quarter) boundaries (~6% pad).
  - Per block: dma_gather 128 xs rows (512B descriptors: full DMA rate), build
    one-hot S[e, d] = (iota == rel) on DVE per overlapped window, and accumulate
    aggT[k, d] += G^T @ S on PE into per-window PSUM ([128,128] x 2 feature halves).
    Self-loops are one identity matmul per window half on contiguous xs_own rows.
  - Group tail: copy aggT to SBUF bf16, pw = aggT^T @ W + sqrt(deg) x b (outer
    product), epilogue out = relu(dinv*pw) - alpha*relu(-dinv*pw), batched out DMA.
"""
import sys
sys.path.insert(0, '/opt/trn_rl_repo')

import numpy as np
import ml_dtypes

N = 100000
NCORES = 8
SH = 12800                 # dst nodes per core
NP = NCORES * SH           # 102400 padded nodes
H = 128                    # output features
KIN = 256                  # input features
WIN = 128                  # dst window size
NW = SH // WIN             # 100 windows per core
WG = 8                     # windows per PSUM group
NG = (NW + WG - 1) // WG   # 13 groups (last has 4)
NQ = 4                     # source quarter tables
QROWS = NP // NQ           # 25600 rows per table (int16-safe)
CB = 32                    # max blocks per dma_gather call

bf16 = ml_dtypes.bfloat16


def _preprocess(edge_index):
    src = np.asarray(edge_index[0]).astype(np.int64)
    dst = np.asarray(edge_index[1]).astype(np.int64)
    E = src.shape[0]

    deg = (np.bincount(dst, minlength=N) + 1).astype(np.float32)
    dinv = (1.0 / np.sqrt(deg)).astype(np.float32)
    dinv_np = np.ones(NP, np.float32)
    dinv_np[:N] = dinv
    sqdeg_np = (1.0 / dinv_np).astype(np.float32)

    core = dst // SH
    w_in_core = (dst % SH) // WIN            # 0..NW-1
    g = w_in_core // WG
    q = src // QROWS
    tabrow = (src % QROWS).astype(np.int64)
    rel_in_grp = (dst % SH - g * (WG * WIN)).astype(np.float32)   # 0..1023

    binid = (core * NG + g) * NQ + q         # per-core bin = (g, q)
    nbins_pc = NG * NQ
    order = np.lexsort((w_in_core, binid))
    o_bin = binid[order]
    o_tab = tabrow[order]
    o_rel = rel_in_grp[order]
    o_core = core[order]

    cnt_all = np.bincount(binid, minlength=NCORES * nbins_pc)
    bin_start = np.concatenate([[0], np.cumsum(cnt_all)])[:-1]
    rank = np.arange(E, dtype=np.int64) - bin_start[o_bin]

    cnt = cnt_all.reshape(NCORES, NG, NQ)
    nblk = np.ceil(cnt.max(axis=0) / WIN).astype(np.int64)        # [NG, NQ]
    pad_sizes = (nblk * WIN).reshape(-1)
    offs = np.concatenate([[0], np.cumsum(pad_sizes)])
    TOT = int(offs[-1])
    NBLK = TOT // WIN

    bin_in_core = o_bin % nbins_pc
    pos = offs[bin_in_core] + rank

    # per-core slot tables
    idx_pc = np.zeros((NCORES, TOT), np.int16)
    rel_pc = np.full((NCORES, TOT), -1.0, np.float32)
    for c in range(NCORES):
        m = o_core == c
        p_c = pos[m]
        idx_pc[c, p_c] = o_tab[m].astype(np.int16)
        rel_pc[c, p_c] = o_rel[m]

    # union window range per block (over cores)
    win_pc = np.where(rel_pc >= 0, rel_pc // WIN, np.nan).reshape(NCORES, NBLK, WIN)
    with np.errstate(all="ignore"):
        wlo = np.nanmin(win_pc, axis=(0, 2)).astype(np.int64)     # [NBLK]
        whi = np.nanmax(win_pc, axis=(0, 2)).astype(np.int64)

    # schedule: groups -> calls (per quarter, <=CB blocks) -> blocks -> pairs
    # pair columns of relp are allocated in issue order.
    groups = []       # per g: dict(calls=[...], last=..)
    pair_cols = []    # per pair: (block_global, wl) to build relp
    last_pair_of_win = {}
    npair = 0
    blk_base = 0
    for gg in range(NG):
        calls = []
        for qq in range(NQ):
            bi = gg * NQ + qq
            nb = int(nblk[gg, qq])
            b0 = int(offs[bi]) // WIN
            done = 0
            while done < nb:
                nbc = min(CB, nb - done)
                blocks = []
                for k in range(nbc):
                    bg = b0 + done + k
                    pairs = []
                    for wl in range(int(wlo[bg]), int(whi[bg]) + 1):
                        w = gg * WG + wl
                        pairs.append((wl, npair))
                        last_pair_of_win[w] = npair
                        pair_cols.append((bg, wl))
                        npair += 1
                    blocks.append((k, pairs))
                calls.append(dict(q=qq, off=int(offs[bi]) + done * WIN,
                                  nidx=nbc * WIN, blocks=blocks))
                done += nbc
        groups.append(calls)
        blk_base += 0
    NPAIR = npair

    # per-core device tensors
    per_core = []
    for c in range(NCORES):
        idx16 = np.tile(np.ascontiguousarray(
            idx_pc[c].reshape(TOT // 16, 16).T), (8, 1))          # [128, TOT//16]
        relp = np.empty((WIN, NPAIR), np.float32)                 # [128, NPAIR]
        relb = rel_pc[c].reshape(NBLK, WIN)
        for p, (bg, wl) in enumerate(pair_cols):
            relp[:, p] = relb[bg] - wl * WIN
        dinv_own = np.ascontiguousarray(
            dinv_np[c * SH:(c + 1) * SH].reshape(NW, WIN).T)      # [128, NW]
        sq_own = np.ascontiguousarray(
            sqdeg_np[c * SH:(c + 1) * SH].astype(bf16)[None, :])  # [1, SH]
        per_core.append(dict(idx16=idx16, relp=np.ascontiguousarray(relp),
                             dinv=dinv_own, dinvn=np.ascontiguousarray(-dinv_own),
                             sq=sq_own))

    sched = dict(groups=groups, last=last_pair_of_win, TOT=TOT, NPAIR=NPAIR)
    return sched, per_core, dinv_np


def _build(sched):
    from concourse import bass, bacc, tile, mybir
    from concourse.masks import make_identity

    nc = bacc.Bacc("TRN2", target_bir_lowering=False, debug=False,
                   enable_asserts=True, num_devices=NCORES)

    TOT = sched["TOT"]
    NPAIR = sched["NPAIR"]
    groups = sched["groups"]
    last = sched["last"]

    xs_q = [nc.dram_tensor(f"xs_q{k}", [QROWS, KIN], mybir.dt.bfloat16,
                           kind="ExternalInput") for k in range(NQ)]
    xso_d = nc.dram_tensor("xs_own", [SH, KIN], mybir.dt.bfloat16, kind="ExternalInput")
    w_d = nc.dram_tensor("w_mat", [KIN, H], mybir.dt.bfloat16, kind="ExternalInput")
    b_d = nc.dram_tensor("b_row", [1, H], mybir.dt.bfloat16, kind="ExternalInput")
    a_d = nc.dram_tensor("a128", [128, H], mybir.dt.float32, kind="ExternalInput")
    dinv_d = nc.dram_tensor("dinv_own", [128, NW], mybir.dt.float32, kind="ExternalInput")
    dinvn_d = nc.dram_tensor("dinvn_own", [128, NW], mybir.dt.float32, kind="ExternalInput")
    sq_d = nc.dram_tensor("sq_own", [1, SH], mybir.dt.bfloat16, kind="ExternalInput")
    idx_d = nc.dram_tensor("idx16", [128, TOT // 16], mybir.dt.int16, kind="ExternalInput")
    relp_d = nc.dram_tensor("relp", [128, NPAIR], mybir.dt.float32, kind="ExternalInput")

    out_d = nc.dram_tensor("out_own", [SH, H], mybir.dt.float32, kind="ExternalOutput")

    with tile.TileContext(nc) as tc:
        with tc.tile_pool(name="consts", bufs=1) as cp, tc.tile_pool(name="sb", bufs=3) as sb:
            iota_i = cp.tile([128, 128], mybir.dt.int32)
            nc.gpsimd.iota(iota_i[:], pattern=[[1, 128]], base=0, channel_multiplier=0)
            iota_b = cp.tile([128, 128], mybir.dt.bfloat16)
            nc.vector.tensor_copy(iota_b[:], iota_i[:])

            ident_f = cp.tile([128, 128], mybir.dt.float32)
            make_identity(nc, ident_f[:])
            ident_b = cp.tile([128, 128], mybir.dt.bfloat16)
            nc.vector.tensor_copy(ident_b[:], ident_f[:])

            w0 = cp.tile([128, H], mybir.dt.bfloat16)
            w1 = cp.tile([128, H], mybir.dt.bfloat16)
            nc.sync.dma_start(w0[:], w_d[0:128, :])
            nc.sync.dma_start(w1[:], w_d[128:256, :])

            dinv_sb = cp.tile([128, NW], mybir.dt.float32)
            nc.sync.dma_start(dinv_sb[:], dinv_d[:, :])
            dinvn_sb = cp.tile([128, NW], mybir.dt.float32)
            nc.sync.dma_start(dinvn_sb[:], dinvn_d[:, :])

            a128 = cp.tile([128, H], mybir.dt.float32)
            nc.sync.dma_start(a128[:], a_d[:, :])
            brow = cp.tile([1, H], mybir.dt.bfloat16)
            nc.sync.dma_start(brow[:], b_d[:, :])
            sq_sb = cp.tile([1, SH], mybir.dt.bfloat16)
            nc.sync.dma_start(sq_sb[:], sq_d[:, :])
            relp_sb = cp.tile([128, NPAIR], mybir.dt.float32)
            nc.sync.dma_start(relp_sb[:], relp_d[:, :])

            with tc.tile_pool(name="psum", bufs=1, space="PSUM") as pp:

                def tail_copies(st):
                    # PSUM -> SBUF copies of the previous group's aggT. Issued at
                    # the top of the next group so its ag banks free up early.
                    for w in range(st["wlo"], st["whi"]):
                        us = []
                        for h in range(2):
                            u = sb.tile([128, 128], mybir.dt.bfloat16, tag="u", bufs=20)
                            nc.scalar.activation(u[:], st["ag"][(w, h)],
                                                 mybir.ActivationFunctionType.Copy)
                            us.append(u)
                        st["us"][w] = us

                def tail_rest(st):
                    # transform + bias + PReLU + store for the previous group; all
                    # dependencies were satisfied a group ago, so these ops do not
                    # stall the sequencers ahead of the current group's work.
                    wlo_t, whi_t = st["wlo"], st["whi"]
                    nwin_t = whi_t - wlo_t
                    o_g = sb.tile([128, WG * H], mybir.dt.float32, tag="o_g", bufs=2)
                    npwb = (nwin_t + 3) // 4
                    pwbank = [pp.tile([128, 512], mybir.dt.float32, space="PSUM",
                                      tag="pw", name=f"pw{wlo_t}_{j}", bufs=2)
                              for j in range(npwb)]
                    for w in range(wlo_t, whi_t):
                        wl = w - wlo_t
                        u0, u1 = st["us"][w]
                        pw = pwbank[wl // 4][:, (wl % 4) * H:(wl % 4) * H + H]
                        pw_first = (wl % 4 == 0)
                        pw_last = (wl == nwin_t - 1) or (wl % 4 == 3)
                        nc.tensor.matmul(out=pw, lhsT=u0[:], rhs=w0[:],
                                         start=pw_first, stop=False)
                        nc.tensor.matmul(out=pw, lhsT=u1[:], rhs=w1[:],
                                         start=False, stop=False)
                        nc.tensor.matmul(out=pw,
                                         lhsT=sq_sb[:, w * WIN:(w + 1) * WIN],
                                         rhs=brow[:], start=False, stop=pw_last)
                        r1 = sb.tile([128, H], mybir.dt.float32, tag="r1", bufs=4)
                        nc.scalar.activation(r1[:], pw,
                                             mybir.ActivationFunctionType.Relu,
                                             scale=dinv_sb[:, w:w + 1])
                        r2 = sb.tile([128, H], mybir.dt.float32, tag="r2", bufs=4)
                        nc.scalar.activation(r2[:], pw,
                                             mybir.ActivationFunctionType.Relu,
                                             scale=dinvn_sb[:, w:w + 1])
                        m = sb.tile([128, H], mybir.dt.float32, tag="m", bufs=4)
                        nc.gpsimd.tensor_tensor(out=m[:], in0=r2[:], in1=a128[:],
                                                op=mybir.AluOpType.mult)
                        nc.vector.tensor_tensor(out=o_g[:, wl * H:(wl + 1) * H],
                                                in0=r1[:], in1=m[:],
                                                op=mybir.AluOpType.subtract)
                    nc.sync.dma_start(
                        out_d[wlo_t * WIN:whi_t * WIN, :].rearrange(
                            "(t p) k -> p t k", p=128),
                        o_g[:, 0:nwin_t * H])

                prev = None
                for gg in range(NG):
                    wlo_g = gg * WG
                    whi_g = min(wlo_g + WG, NW)
                    nwin = whi_g - wlo_g

                    # group g's loads first (SP), ahead of group g-1's store
                    xso_g = sb.tile([128, WG, KIN], mybir.dt.bfloat16, tag="xso", bufs=2)
                    nc.sync.dma_start(
                        xso_g[:, 0:nwin, :],
                        xso_d[wlo_g * WIN:whi_g * WIN, :].rearrange(
                            "(t p) k -> p t k", p=128))
                    idx_tiles = []
                    for call in groups[gg]:
                        off, nidx = call["off"], call["nidx"]
                        it = sb.tile([128, CB * 8], mybir.dt.int16, tag="idx", bufs=10)
                        nc.sync.dma_start(it[:, 0:nidx // 16],
                                          idx_d[:, off // 16:(off + nidx) // 16])
                        idx_tiles.append(it)

                    # free previous group's ag banks
                    if prev is not None:
                        tail_copies(prev)

                    # aggT regions packed 4-per-PSUM-bank: tile [128,512] fp32.
                    # One accumulation group per 2KB zero region (= bank): only the
                    # first matmul issued into a bank starts (zeroing the whole
                    # bank), only the last one stops.
                    nbk = (2 * nwin + 3) // 4
                    agbank = [pp.tile([128, 512], mybir.dt.float32, space="PSUM",
                                      tag="ag", name=f"ag{gg}_{j}", bufs=6)
                              for j in range(nbk)]
                    bank_stop = {}     # bank j -> pcol of its stop pair (or None)
                    for j in range(nbk):
                        ws = [wlo_g + r // 2 for r in range(j * 4, min(j * 4 + 4, 2 * nwin))]
                        pcs = [last[w] for w in set(ws) if w in last]
                        bank_stop[j] = max(pcs) if pcs else None
                    ag = {}
                    bank_of = {}
                    for w in range(wlo_g, whi_g):
                        wl = w - wlo_g
                        for h in range(2):
                            ridx = wl * 2 + h
                            j = ridx // 4
                            ag[(w, h)] = agbank[j][:, (ridx % 4) * 128:
                                                   (ridx % 4) * 128 + 128]
                            bank_of[(w, h)] = j
                            is_last_self = (bank_stop[j] is None
                                            and ridx == min(j * 4 + 3, 2 * nwin - 1))
                            nc.tensor.matmul(
                                out=ag[(w, h)],
                                lhsT=xso_g[:, wl, h * 128:(h + 1) * 128],
                                rhs=ident_b[:],
                                start=(ridx % 4 == 0), stop=is_last_self)

                    for ci, call in enumerate(groups[gg]):
                        qq, off, nidx = call["q"], call["off"], call["nidx"]
                        nbc = nidx // WIN
                        idx_sb = idx_tiles[ci]
                        g_t = sb.tile([128, CB, KIN], mybir.dt.bfloat16, tag="g_t", bufs=3)
                        nc.gpsimd.dma_gather(
                            g_t[:, 0:nbc, :], xs_q[qq][:, :],
                            idx_sb[:, 0:nidx // 16], nidx, nidx, KIN,
                            single_packet=False)
                        for (slot, pairs) in call["blocks"]:
                            for (wl, pcol) in pairs:
                                w = wlo_g + wl
                                s_t = sb.tile([128, 128], mybir.dt.bfloat16,
                                              tag="s_t", bufs=8)
                                nc.vector.tensor_scalar(
                                    out=s_t[:], in0=iota_b[:],
                                    scalar1=relp_sb[:, pcol:pcol + 1], scalar2=None,
                                    op0=mybir.AluOpType.is_equal)
                                for h in range(2):
                                    nc.tensor.matmul(
                                        out=ag[(w, h)],
                                        lhsT=g_t[:, slot, h * 128:(h + 1) * 128],
                                        rhs=s_t[:],
                                        start=False,
                                        stop=(bank_stop[bank_of[(w, h)]] == pcol
                                              and h == 1))
                        if ci == 0 and prev is not None:
                            tail_rest(prev)
                            prev = None
                    if prev is not None:   # group with no calls (unlikely)
                        tail_rest(prev)
                    prev = dict(wlo=wlo_g, whi=whi_g, ag=ag, us={})

                # drain the last group
                tail_copies(prev)
                tail_rest(prev)

    nc.compile()
    return nc


_LAST = {}


def kernel(x, edge_index, W, b, alpha):
    from concourse.bass_utils import run_bass_kernel_spmd

    x = np.asarray(x, dtype=np.float32)
    W = np.asarray(W, dtype=np.float32)
    b = np.asarray(b, dtype=np.float32)
    alpha = np.asarray(alpha, dtype=np.float32)

    sched, per_core, dinv_np = _preprocess(edge_index)
    nc = _build(sched)
    _LAST["nc"] = nc
    _LAST["sched"] = sched

    xs_pad = np.zeros((NP, KIN), np.float32)
    xs_pad[:N] = x * dinv_np[:N, None]
    xs_bf = xs_pad.astype(bf16)
    xsq = [np.ascontiguousarray(xs_bf[k * QROWS:(k + 1) * QROWS]) for k in range(NQ)]

    W_bf = W.astype(bf16)
    b_row = b.astype(bf16)[None, :]
    a128 = np.tile(alpha[None, :], (128, 1)).astype(np.float32)

    in_maps = []
    for c in range(NCORES):
        in_maps.append({
            "xs_q0": xsq[0], "xs_q1": xsq[1], "xs_q2": xsq[2], "xs_q3": xsq[3],
            "xs_own": np.ascontiguousarray(xs_bf[c * SH:(c + 1) * SH]),
            "w_mat": W_bf, "b_row": b_row, "a128": a128,
            "dinv_own": per_core[c]["dinv"], "dinvn_own": per_core[c]["dinvn"],
            "sq_own": per_core[c]["sq"],
            "idx16": per_core[c]["idx16"], "relp": per_core[c]["relp"],
        })

    res = run_bass_kernel_spmd(nc, in_maps, core_ids=list(range(NCORES)))
    out = np.concatenate([res.results[c]["out_own"] for c in range(NCORES)], axis=0)
    return np.ascontiguousarray(out[:N])
